# revision 32
# baseline (speedup 1.0000x reference)
"""CGRU cell on 8 Trainium2 NeuronCores.

Strategy: data-parallel over the batch dim (4096 -> 8 x 512), zero
cross-core communication, weights replicated.  On-core compute runs in
transposed space ([feature, batch]): W tiles are the stationary matmul
operand, x^T/h^T tiles [128, 512] the moving operand.

The complex "cat kernel" [[R, -I], [I, R]] is evaluated with Gauss's
3-multiplication trick instead of 4 real matmul chains:
    k1   = (xr + xi) @ R        (shared by both outputs)
    real = k1 + xi @ (I - R)
    imag = k1 + xr @ (-I - R)
which cuts PE work by 25%.  The combine (k1 + A) is a DVE
scalar_tensor_tensor (k1 staged to SBUF first - ALU ops may read only
one PSUM operand), fused with the per-feature bias; hard-sigmoid
scale/clip ride the scalar-engine activation.

The r gate (least error-sensitive: its error passes through another
matmul and a tanh) runs in fp8-e4m3 with DoubleRow perf mode: each
matmul contracts TWO k-tiles in the same 216 ns a fp16 matmul spends on
one (157 TF/s).  Its weights are host-scaled by S8=64 to stay out of
the fp8 subnormal range; the activation's scale folds 0.2/S8 back out.
z and hh stay fp16: fp8 there breaks the 2e-2 error budget (measured).

Phase order r -> z -> hh: the r phase's inputs (host-quantized fp8 x/h
and their sums) total ~3 MB, so the PE starts almost immediately while
the 20+ MB of fp16 activations/weights stream in its shadow - the
startup is otherwise HBM-bound.  fp16 triples stream R/(I-R) per half
and derive -(I+R) = -2R - (I-R) on the DVE, saving a third of the
weight DMA.  Everything accumulates in fp32 PSUM; outputs store as
fp16 (upcast on host).
"""

import numpy as np
import ml_dtypes

import concourse.bass as bass
import concourse.mybir as mybir
import concourse.tile as tile
from concourse import bacc
from concourse.bass_utils import run_bass_kernel_spmd

B, D, U = 4096, 1024, 1024
NCORES = 8
N = B // NCORES          # batch rows per core (moving free dim)
P = 128                  # partition size
KT = D // P              # 8 k-tiles per complex half
MT = U // P              # 8 m-tiles per complex half
F = 2 * U                # 2048 features
MCOLS = KT * P           # 1024 cols per per-matrix weight tile
S8 = 64.0                # fp8 weight pre-scale for the r gate
WARM = 22

F16 = mybir.dt.float16
F32 = mybir.dt.float32
F8 = mybir.dt.float8e4
AF = mybir.ActivationFunctionType
OP = mybir.AluOpType
DR = mybir.MatmulPerfMode.DoubleRow

_CACHE = {}


def _build():
    nc = bacc.Bacc("TRN2", target_bir_lowering=False, debug=False)

    xT = nc.dram_tensor("xT", [F, N], F16, kind="ExternalInput")
    hT = nc.dram_tensor("hT", [F, N], F16, kind="ExternalInput")
    x8d = nc.dram_tensor("x8d", [F, N], F8, kind="ExternalInput")
    h8d = nc.dram_tensor("h8d", [F, N], F8, kind="ExternalInput")
    xs8d = nc.dram_tensor("xs8d", [D, N], F8, kind="ExternalInput")
    hs8d = nc.dram_tensor("hs8d", [D, N], F8, kind="ExternalInput")
    # z/hh weights: [gate, m, mat, part, k*128] fp16,
    # mats: R, I-R, -(I+R), RR, IR-RR, -(IR+RR)
    w16 = nc.dram_tensor("w16", [2, MT, 6, P, MCOLS], F16, kind="ExternalInput")
    # r weights: same six mats, * S8, [m, mat, part, k, 128] fp8 (DoubleRow)
    w8 = nc.dram_tensor("w8", [MT, 6, P, KT, P], F8, kind="ExternalInput")
    bz = nc.dram_tensor("bz", [P, 2 * MT], F32, kind="ExternalInput")
    br = nc.dram_tensor("br", [P, 2 * MT], F32, kind="ExternalInput")
    bh = nc.dram_tensor("bh", [P, 2 * MT], F32, kind="ExternalInput")
    wz = nc.dram_tensor("wz", [P, P + N], F16, kind="ExternalInput")
    oT = nc.dram_tensor("oT", [F, N], F16, kind="ExternalOutput")

    with tile.TileContext(nc) as tc:
        with (
            tc.tile_pool(name="res", bufs=1) as res,
            tc.tile_pool(name="wts", bufs=18) as wts,
            tc.tile_pool(name="w8p", bufs=18) as w8p,
            tc.tile_pool(name="act", bufs=4) as act,
            tc.tile_pool(name="ps", bufs=7, space="PSUM") as psp,
            tc.tile_pool(name="wm", bufs=1, space="PSUM") as wmp,
        ):
            # PE warmup: keeps the HAM activity window busy while the
            # first DMAs land.  Warm operands come from a tiny DMA (the
            # DVE isn't up until ~7us, a DMA lands by ~4us).
            wsrc = res.tile([P, P], F16, tag="wsrc")
            dmov = res.tile([P, N], F16, tag="dmov")
            nc.sync.dma_start(wsrc[:], wz[:, 0:P])
            nc.sync.dma_start(dmov[:], wz[:, P:P + N])
            wps = wmp.tile([P, N], F32, tag="warm")
            for _ in range(WARM):
                nc.tensor.matmul(wps[:], wsrc[:], dmov[:], start=True, stop=True)

            xs = res.tile([P, 2 * MT, N], F16, tag="xs")      # xr 0..7, xi 8..15
            hs = res.tile([P, 2 * MT, N], F16, tag="hs")
            xsum = res.tile([P, KT, N], F16, tag="xsum")      # xr + xi
            hsum = res.tile([P, KT, N], F16, tag="hsum")
            x8 = res.tile([P, 2 * MT, N], F8, tag="x8")
            h8 = res.tile([P, 2 * MT, N], F8, tag="h8")
            xsum8 = res.tile([P, KT, N], F8, tag="xsum8")
            hsum8 = res.tile([P, KT, N], F8, tag="hsum8")
            rh = res.tile([P, 2 * MT, N], F16, tag="rh")      # min(r,1)*h
            rhsum = res.tile([P, KT, N], F16, tag="rhsum")
            zs = res.tile([P, 2 * MT, N], F16, tag="zs")
            bz_sb = res.tile([P, 2 * MT], F32, tag="bz")
            br_sb = res.tile([P, 2 * MT], F32, tag="br")
            bh_sb = res.tile([P, 2 * MT], F32, tag="bh")

            nc.scalar.dma_start(br_sb[:], br[:])
            nc.scalar.dma_start(bz_sb[:], bz[:])
            nc.scalar.dma_start(bh_sb[:], bh[:])

            def wload(ws, gi, m, j):
                wt = wts.tile([P, MCOLS], F16, tag="w")
                nc.sync.dma_start(wt[:], w16[gi, m, j])
                ws[j] = wt

            def w8load(ws, m, j):
                wt = w8p.tile([P, KT, P], F8, tag="w8")
                nc.sync.dma_start(wt[:], w8[m, j])
                ws[j] = wt

            def ld(dst, j, src):
                nc.sync.dma_start(dst[:, j, :], src[j * P:(j + 1) * P, :])

            # --- startup: one JIT-ordered DMA stream on the sync queue.
            # The fp8 r-phase operands and r0/r1 weights go first (the PE
            # starts on them ~7us in); everything fp16 streams behind. ---
            ws_r = [[None] * 6 for _ in range(MT)]
            for k in range(4):
                ld(x8, MT + k, x8d)
                ld(x8, k, x8d)
            w8load(ws_r[0], 0, 1)
            w8load(ws_r[0], 0, 2)
            for k in range(4, KT):
                ld(x8, MT + k, x8d)
                ld(x8, k, x8d)
            w8load(ws_r[0], 0, 0)
            for k in range(KT):
                ld(xsum8, k, xs8d)
            w8load(ws_r[1], 1, 1)
            w8load(ws_r[1], 1, 2)
            w8load(ws_r[1], 1, 0)
            for k in range(4):
                ld(h8, MT + k, h8d)
                ld(h8, k, h8d)
            w8load(ws_r[0], 0, 4)
            w8load(ws_r[0], 0, 3)
            for k in range(4, KT):
                ld(h8, MT + k, h8d)
                ld(h8, k, h8d)
            w8load(ws_r[0], 0, 5)
            for k in range(KT):
                ld(hsum8, k, hs8d)
            w8load(ws_r[1], 1, 4)
            w8load(ws_r[1], 1, 3)
            w8load(ws_r[1], 1, 5)
            # remaining r weights interleaved with fp16 h (needed by the
            # rh combines from ~13us on, column-m at a time)
            for m in range(2, MT):
                for j in (1, 2, 0, 4, 3, 5):
                    w8load(ws_r[m], m, j)
                ld(hs, MT + (m - 2), hT)
                ld(hs, m - 2, hT)
            for k in range(MT - 2, KT):
                ld(hs, MT + k, hT)
                ld(hs, k, hT)
            # fp16 x streams behind everything the r phase needs
            for k in range(KT):
                ld(xs, MT + k, xT)
                ld(xs, k, xT)

            class T:
                def __init__(self, kind, m, cols, ws=None, tail=False):
                    self.kind, self.m, self.cols, self.tail = kind, m, cols, tail
                    # alloc order (A, B, K1) matches in-half issue order so
                    # bank recycling waits line up with the combine stts
                    self.A = psp.tile([P, N], F32, tag="ps")
                    self.B = psp.tile([P, N], F32, tag="ps")
                    self.K1 = psp.tile([P, N], F32, tag="ps")
                    if ws is not None:
                        self.ws = ws
                        return
                    if kind == "r":
                        self.ws = [None] * 6
                        for j in (1, 2, 0, 4, 3, 5):
                            w8load(self.ws, m, j)
                    else:
                        # stream R/(I-R) per half, derive -(I+R) = -2R-(I-R)
                        # on the DVE (saves a third of the weight DMA)
                        gi = 0 if kind == "z" else 1
                        self.ws = [None] * 6
                        for base in (0, 3):
                            wload(self.ws, gi, m, base + 1)
                            wload(self.ws, gi, m, base)
                            wb = wts.tile([P, MCOLS], F16, tag="w")
                            nc.vector.scalar_tensor_tensor(
                                wb[:], self.ws[base][:], -2.0,
                                self.ws[base + 1][:],
                                op0=OP.mult, op1=OP.subtract)
                            self.ws[base + 2] = wb

            def dr_chain(t, ps, wj, buf, off, start):
                c = t.cols
                for j in range(KT // 2):
                    nc.tensor.matmul(
                        ps[:, c], t.ws[wj][:, 2 * j:2 * j + 2, :],
                        buf[:, off + 2 * j:off + 2 * j + 2, c],
                        start=(start and j == 0),
                        stop=(not start and j == KT // 2 - 1),
                        perf_mode=DR)

            def f16_chain(t, ps, wj, buf, off, start):
                c = t.cols
                for k in range(KT):
                    nc.tensor.matmul(
                        ps[:, c], t.ws[wj][:, k * P:(k + 1) * P],
                        buf[:, off + k, c],
                        start=(start and k == 0),
                        stop=(not start and k == KT - 1))

            def in_half(t, kmajor=False):
                c = t.cols
                if t.kind == "r":
                    if kmajor:
                        # consume (xi8[2j..], xr8[2j..]) at DMA arrival rate
                        for j in range(KT // 2):
                            nc.tensor.matmul(
                                t.A[:, c], t.ws[1][:, 2 * j:2 * j + 2, :],
                                x8[:, MT + 2 * j:MT + 2 * j + 2, c],
                                start=(j == 0), stop=False, perf_mode=DR)
                            nc.tensor.matmul(
                                t.B[:, c], t.ws[2][:, 2 * j:2 * j + 2, :],
                                x8[:, 2 * j:2 * j + 2, c],
                                start=(j == 0), stop=False, perf_mode=DR)
                        dr_chain(t, t.K1, 0, xsum8, 0, True)
                    else:
                        dr_chain(t, t.A, 1, x8, MT, True)
                        dr_chain(t, t.B, 2, x8, 0, True)
                        dr_chain(t, t.K1, 0, xsum8, 0, True)
                else:
                    f16_chain(t, t.A, 1, xs, MT, True)
                    f16_chain(t, t.B, 2, xs, 0, True)
                    f16_chain(t, t.K1, 0, xsum, 0, True)

            def rec_and_combine(t):
                c, m = t.cols, t.m
                if t.kind == "r":
                    dr_chain(t, t.A, 4, h8, MT, False)
                    dr_chain(t, t.K1, 3, hsum8, 0, False)
                else:
                    mv, sm = (rh, rhsum) if t.kind == "h" else (hs, hsum)
                    f16_chain(t, t.A, 4, mv, MT, False)
                    f16_chain(t, t.K1, 3, sm, 0, False)
                # ALU ops may read only one PSUM operand: stage K1 in SBUF
                # (also releases its bank early)
                k1sb = act.tile([P, N], F32, tag="k1")
                nc.scalar.copy(k1sb[:, c], t.K1[:, c])
                pre_r = act.tile([P, N], F16, tag="pre")
                nc.vector.scalar_tensor_tensor(
                    pre_r[:, c], k1sb[:, c], 1.0, t.A[:, c],
                    op0=OP.mult, op1=OP.add)
                if t.kind == "r":
                    dr_chain(t, t.B, 5, h8, 0, False)
                else:
                    f16_chain(t, t.B, 5, mv, 0, False)
                pre_i = act.tile([P, N], F16, tag="pre")
                for ch in ((slice(0, N // 2), slice(N // 2, N))
                           if t.tail else (c,)):
                    nc.vector.scalar_tensor_tensor(
                        pre_i[:, ch], k1sb[:, ch], 1.0, t.B[:, ch],
                        op0=OP.mult, op1=OP.add)

                if t.kind == "z":
                    for col, pre in ((m, pre_r), (MT + m, pre_i)):
                        nc.scalar.activation(zs[:, col, c], pre[:, c], AF.Relu,
                                             bias=bz_sb[:, col:col + 1],
                                             scale=0.2)
                elif t.kind == "r":
                    for col, pre in ((m, pre_r), (MT + m, pre_i)):
                        rr = act.tile([P, N], F16, tag="rr")
                        nc.scalar.activation(rr[:, c], pre[:, c], AF.Relu,
                                             bias=br_sb[:, col:col + 1],
                                             scale=0.2 / S8)
                        nc.vector.scalar_tensor_tensor(
                            rh[:, col, c], rr[:, c], 1.0, hs[:, col, c],
                            op0=OP.min, op1=OP.mult)
                    nc.vector.tensor_tensor(rhsum[:, m, c], rh[:, m, c],
                                            rh[:, MT + m, c], OP.add)
                elif t.tail:
                    # short tail: o = u + w*t with u = min(z,1)*h and
                    # w = relu(1-z) precomputed off the critical path
                    for part, (col, pre) in enumerate(((m, pre_r),
                                                       (MT + m, pre_i))):
                        chunks = ((slice(0, N // 2), slice(N // 2, N))
                                  if part else (c,))
                        for ch in chunks:
                            t_ = act.tile([P, N], F16, tag="t")
                            nc.scalar.activation(t_[:, ch], pre[:, ch],
                                                 AF.Tanh,
                                                 bias=bh_sb[:, col:col + 1])
                            v = act.tile([P, N], F16, tag="d")
                            nc.vector.tensor_tensor(
                                v[:, ch], wpre[part][:, ch], t_[:, ch],
                                OP.mult)
                            o = act.tile([P, N], F16, tag="o")
                            nc.vector.tensor_tensor(o[:, ch], v[:, ch],
                                                    upre[part][:, ch], OP.add)
                            nc.sync.dma_start(oT[col * P:(col + 1) * P, ch],
                                              o[:, ch])
                else:
                    for col, pre in ((m, pre_r), (MT + m, pre_i)):
                        t_ = act.tile([P, N], F16, tag="t")
                        nc.scalar.activation(t_[:, c], pre[:, c], AF.Tanh,
                                             bias=bh_sb[:, col:col + 1])
                        d = act.tile([P, N], F16, tag="d")
                        nc.vector.scalar_tensor_tensor(
                            d[:, c], t_[:, c], -1.0, hs[:, col, c],
                            op0=OP.mult, op1=OP.add)
                        e = act.tile([P, N], F16, tag="e")
                        nc.vector.scalar_tensor_tensor(
                            e[:, c], zs[:, col, c], 1.0, d[:, c],
                            op0=OP.min, op1=OP.mult)
                        o = act.tile([P, N], F16, tag="o")
                        nc.vector.tensor_tensor(o[:, c], e[:, c], t_[:, c],
                                                OP.add)
                        nc.sync.dma_start(oT[col * P:(col + 1) * P, c],
                                          o[:, c])

            upre = [res.tile([P, N], F16, tag="u0", name="u0"),
                    res.tile([P, N], F16, tag="u1", name="u1")]
            wpre = [res.tile([P, N], F16, tag="w0", name="w0"),
                    res.tile([P, N], F16, tag="w1", name="w1")]

            full = slice(0, N)
            plan = ([("r", m) for m in range(MT)]
                    + [("z", m) for m in range(MT)]
                    + [("h", m) for m in range(MT)])

            prev = None
            for kind, m in plan:
                if (kind, m) == ("z", 0):
                    # sums for the fp16 k1 chains; emitted here so the DVE
                    # doesn't block the r combines while x/h stream in
                    for k in range(KT):
                        nc.vector.tensor_tensor(xsum[:, k, :], xs[:, k, :],
                                                xs[:, MT + k, :], OP.add)
                    for k in range(KT):
                        nc.vector.tensor_tensor(hsum[:, k, :], hs[:, k, :],
                                                hs[:, MT + k, :], OP.add)
                tail = kind == "h" and m == MT - 1
                if tail:
                    for part, col in enumerate((MT - 1, 2 * MT - 1)):
                        nc.vector.scalar_tensor_tensor(
                            upre[part][:], zs[:, col, :], 1.0, hs[:, col, :],
                            op0=OP.min, op1=OP.mult)
                        nc.scalar.activation(wpre[part][:], zs[:, col, :],
                                             AF.Relu, bias=1.0, scale=-1.0)
                pre_ws = ws_r[m] if kind == "r" else None
                t = T(kind, m, full, ws=pre_ws, tail=tail)
                in_half(t, kmajor=(prev is None))
                if prev is not None:
                    rec_and_combine(prev)
                prev = t
            rec_and_combine(prev)

    nc.compile()
    return nc


def _tiles(mat):
    # (1024, 1024) -> [m, k, 128, 128] tile array
    return mat.reshape(KT, P, MT, P).transpose(2, 0, 1, 3)


def _gate_mats(real_kernel, imaginary_kernel, real_recurrent_kernel,
               imaginary_recurrent_kernel, g):
    def gate(Wmat):
        return np.asarray(Wmat[:, g * U:(g + 1) * U], dtype=np.float32)

    R, I = gate(real_kernel), gate(imaginary_kernel)
    RR, IR = gate(real_recurrent_kernel), gate(imaginary_recurrent_kernel)
    return [R, I - R, -(I + R), RR, IR - RR, -(IR + RR)]


def _blob16(mats):
    arr = np.stack([_tiles(m) for m in mats])   # [6, m, k, p, c]
    arr = arr.transpose(1, 0, 3, 2, 4)          # [m, mat, p, k, c]
    return arr.reshape(MT, 6, P, MCOLS).astype(np.float16)


def _blob8(mats):
    arr = np.stack([_tiles(m) for m in mats])
    arr = arr.transpose(1, 0, 3, 2, 4) * S8     # [m, mat, p, k, c]
    return np.ascontiguousarray(arr).astype(ml_dtypes.float8_e4m3)


def prepare_in_maps(inputs, h_tm1, real_kernel, imaginary_kernel,
                    real_recurrent_kernel, imaginary_recurrent_kernel,
                    real_bias, imaginary_bias):
    inputs = np.asarray(inputs, dtype=np.float32)
    h_tm1 = np.asarray(h_tm1, dtype=np.float32)

    def mats(g):
        return _gate_mats(real_kernel, imaginary_kernel,
                          real_recurrent_kernel, imaginary_recurrent_kernel, g)

    w16_np = np.ascontiguousarray(np.stack([_blob16(mats(0)),
                                            _blob16(mats(2))]))
    w8_np = _blob8(mats(1))

    def cat_bias(g):
        return np.concatenate([
            np.asarray(real_bias[g * U:(g + 1) * U], dtype=np.float32),
            np.asarray(imaginary_bias[g * U:(g + 1) * U], dtype=np.float32),
        ])

    def pcols(v):  # [2U] -> [P, 2MT]
        return np.ascontiguousarray(v.reshape(2 * MT, P).T)

    bz_np = pcols(0.2 * cat_bias(0) + 0.5)
    br_np = pcols(0.2 * cat_bias(1) + 0.5)
    bh_np = pcols(cat_bias(2))
    wz_np = np.zeros((P, P + N), dtype=np.float16)

    in_maps = []
    for c in range(NCORES):
        sl = slice(c * N, (c + 1) * N)
        xT = inputs[sl].T.astype(np.float16)
        hT = h_tm1[sl].T.astype(np.float16)
        xsd = (inputs[sl, :D].T + inputs[sl, D:].T).astype(np.float16)
        hsd = (h_tm1[sl, :D].T + h_tm1[sl, D:].T).astype(np.float16)
        in_maps.append({
            "xT": xT, "hT": hT,
            "x8d": xT.astype(ml_dtypes.float8_e4m3),
            "h8d": hT.astype(ml_dtypes.float8_e4m3),
            "xs8d": xsd.astype(ml_dtypes.float8_e4m3),
            "hs8d": hsd.astype(ml_dtypes.float8_e4m3),
            "w16": w16_np, "w8": w8_np,
            "bz": bz_np, "br": br_np, "bh": bh_np, "wz": wz_np,
        })
    return in_maps


def get_nc():
    if "nc" not in _CACHE:
        _CACHE["nc"] = _build()
    return _CACHE["nc"]


def gather(results):
    out = np.empty((B, F), dtype=np.float32)
    for c in range(NCORES):
        out[c * N:(c + 1) * N] = results[c]["oT"].T.astype(np.float32)
    return out


def kernel(**inputs):
    nc = get_nc()
    in_maps = prepare_in_maps(**inputs)
    res = run_bass_kernel_spmd(nc, in_maps, list(range(NCORES)))
    return gather(res.results)


# revision 33
# speedup vs baseline: 1.1902x; 1.1902x over previous
"""CGRU cell on 8 Trainium2 NeuronCores.

Strategy: data-parallel over the batch dim (4096 -> 8 x 512), zero
cross-core communication, weights replicated.  On-core compute runs in
transposed space ([feature, batch]): W tiles are the stationary matmul
operand, x^T/h^T tiles [128, 512] the moving operand.

The complex "cat kernel" [[R, -I], [I, R]] is evaluated with Gauss's
3-multiplication trick instead of 4 real matmul chains:
    k1   = (xr + xi) @ R        (shared by both outputs)
    real = k1 + xi @ (I - R)
    imag = k1 + xr @ (-I - R)
which cuts PE work by 25%.  The combine (k1 + A) is a DVE
scalar_tensor_tensor (k1 staged to SBUF first - ALU ops may read only
one PSUM operand), fused with the per-feature bias; hard-sigmoid
scale/clip ride the scalar-engine activation.

The r gate (least error-sensitive: its error passes through another
matmul and a tanh) runs in fp8-e4m3 with DoubleRow perf mode: each
matmul contracts TWO k-tiles in the same 216 ns a fp16 matmul spends on
one (157 TF/s).  Its weights are host-scaled by S8=64 to stay out of
the fp8 subnormal range; the activation's scale folds 0.2/S8 back out.
z and hh stay fp16: fp8 there breaks the 2e-2 error budget (measured).

Phase order r -> z -> hh: the r phase's inputs (host-quantized fp8 x/h
and their sums) total ~3 MB, so the PE starts almost immediately while
the 20+ MB of fp16 activations/weights stream in its shadow - the
startup is otherwise HBM-bound.  fp16 triples stream R/(I-R) per half
and derive -(I+R) = -2R - (I-R) on the DVE, saving a third of the
weight DMA.  Everything accumulates in fp32 PSUM; outputs store as
fp16 (upcast on host).
"""

import numpy as np
import ml_dtypes

import concourse.bass as bass
import concourse.mybir as mybir
import concourse.tile as tile
from concourse import bacc
from concourse.bass_utils import run_bass_kernel_spmd

B, D, U = 4096, 1024, 1024
NCORES = 8
N = B // NCORES          # batch rows per core (moving free dim)
P = 128                  # partition size
KT = D // P              # 8 k-tiles per complex half
MT = U // P              # 8 m-tiles per complex half
F = 2 * U                # 2048 features
MCOLS = KT * P           # 1024 cols per per-matrix weight tile
S8 = 64.0                # fp8 weight pre-scale for the r gate
WARM = 14

F16 = mybir.dt.float16
F32 = mybir.dt.float32
F8 = mybir.dt.float8e4
AF = mybir.ActivationFunctionType
OP = mybir.AluOpType
DR = mybir.MatmulPerfMode.DoubleRow

_CACHE = {}


def _build():
    nc = bacc.Bacc("TRN2", target_bir_lowering=False, debug=False)

    xT = nc.dram_tensor("xT", [F, N], F16, kind="ExternalInput")
    hT = nc.dram_tensor("hT", [F, N], F16, kind="ExternalInput")
    x8d = nc.dram_tensor("x8d", [F, N], F8, kind="ExternalInput")
    h8d = nc.dram_tensor("h8d", [F, N], F8, kind="ExternalInput")
    xs8d = nc.dram_tensor("xs8d", [D, N], F8, kind="ExternalInput")
    hs8d = nc.dram_tensor("hs8d", [D, N], F8, kind="ExternalInput")
    # z/hh weights: [gate, m, mat, part, k*128] fp16,
    # mats: R, I-R, -(I+R), RR, IR-RR, -(IR+RR)
    w16 = nc.dram_tensor("w16", [2, MT, 6, P, MCOLS], F16, kind="ExternalInput")
    # r weights: same six mats, * S8, [m, mat, part, k, 128] fp8 (DoubleRow)
    w8 = nc.dram_tensor("w8", [MT, 6, P, KT, P], F8, kind="ExternalInput")
    bz = nc.dram_tensor("bz", [P, 2 * MT], F32, kind="ExternalInput")
    br = nc.dram_tensor("br", [P, 2 * MT], F32, kind="ExternalInput")
    bh = nc.dram_tensor("bh", [P, 2 * MT], F32, kind="ExternalInput")
    wz = nc.dram_tensor("wz", [P, P + N], F16, kind="ExternalInput")
    oT = nc.dram_tensor("oT", [F, N], F16, kind="ExternalOutput")

    with tile.TileContext(nc) as tc:
        with (
            tc.tile_pool(name="res", bufs=1) as res,
            tc.tile_pool(name="wts", bufs=18) as wts,
            tc.tile_pool(name="w8p", bufs=18) as w8p,
            tc.tile_pool(name="act", bufs=4) as act,
            tc.tile_pool(name="ps", bufs=7, space="PSUM") as psp,
            tc.tile_pool(name="wm", bufs=1, space="PSUM") as wmp,
        ):
            # PE warmup: keeps the HAM activity window busy while the
            # first DMAs land.  Warm operands come from a tiny DMA (the
            # DVE isn't up until ~7us, a DMA lands by ~4us).
            wsrc = res.tile([P, P], F16, tag="wsrc")
            dmov = res.tile([P, N], F16, tag="dmov")
            nc.sync.dma_start(wsrc[:], wz[:, 0:P])
            nc.sync.dma_start(dmov[:], wz[:, P:P + N])
            wps = wmp.tile([P, N], F32, tag="warm")
            for _ in range(WARM):
                nc.tensor.matmul(wps[:], wsrc[:], dmov[:], start=True, stop=True)

            xs = res.tile([P, 2 * MT, N], F16, tag="xs")      # xr 0..7, xi 8..15
            hs = res.tile([P, 2 * MT, N], F16, tag="hs")
            xsum = res.tile([P, KT, N], F16, tag="xsum")      # xr + xi
            hsum = res.tile([P, KT, N], F16, tag="hsum")
            x8 = res.tile([P, 2 * MT, N], F8, tag="x8")
            h8 = res.tile([P, 2 * MT, N], F8, tag="h8")
            xsum8 = res.tile([P, KT, N], F8, tag="xsum8")
            hsum8 = res.tile([P, KT, N], F8, tag="hsum8")
            rh = res.tile([P, 2 * MT, N], F16, tag="rh")      # min(r,1)*h
            rhsum = res.tile([P, KT, N], F16, tag="rhsum")
            zs = res.tile([P, 2 * MT, N], F16, tag="zs")
            bz_sb = res.tile([P, 2 * MT], F32, tag="bz")
            br_sb = res.tile([P, 2 * MT], F32, tag="br")
            bh_sb = res.tile([P, 2 * MT], F32, tag="bh")

            nc.scalar.dma_start(br_sb[:], br[:])
            nc.scalar.dma_start(bz_sb[:], bz[:])
            nc.scalar.dma_start(bh_sb[:], bh[:])

            def wload(ws, gi, m, j):
                wt = wts.tile([P, MCOLS], F16, tag="w")
                nc.sync.dma_start(wt[:], w16[gi, m, j])
                ws[j] = wt

            def w8load(ws, m, j):
                wt = w8p.tile([P, KT, P], F8, tag="w8")
                nc.sync.dma_start(wt[:], w8[m, j])
                ws[j] = wt

            def ld(dst, j, src):
                nc.sync.dma_start(dst[:, j, :], src[j * P:(j + 1) * P, :])

            # --- startup: one JIT-ordered DMA stream on the sync queue.
            # The fp8 r-phase operands and r0/r1 weights go first (the PE
            # starts on them ~7us in); everything fp16 streams behind. ---
            ws_r = [[None] * 6 for _ in range(MT)]
            for k in range(4):
                ld(x8, MT + k, x8d)
                ld(x8, k, x8d)
            w8load(ws_r[0], 0, 1)
            w8load(ws_r[0], 0, 2)
            for k in range(4, KT):
                ld(x8, MT + k, x8d)
                ld(x8, k, x8d)
            w8load(ws_r[0], 0, 0)
            for k in range(KT):
                ld(xsum8, k, xs8d)
            w8load(ws_r[1], 1, 1)
            w8load(ws_r[1], 1, 2)
            w8load(ws_r[1], 1, 0)
            for k in range(4):
                ld(h8, MT + k, h8d)
                ld(h8, k, h8d)
            w8load(ws_r[0], 0, 4)
            w8load(ws_r[0], 0, 3)
            for k in range(4, KT):
                ld(h8, MT + k, h8d)
                ld(h8, k, h8d)
            w8load(ws_r[0], 0, 5)
            for k in range(KT):
                ld(hsum8, k, hs8d)
            w8load(ws_r[1], 1, 4)
            w8load(ws_r[1], 1, 3)
            w8load(ws_r[1], 1, 5)
            # remaining r weights interleaved with fp16 h (needed by the
            # rh combines from ~13us on, column-m at a time)
            for m in range(2, MT):
                for j in (1, 2, 0, 4, 3, 5):
                    w8load(ws_r[m], m, j)
                ld(hs, MT + (m - 2), hT)
                ld(hs, m - 2, hT)
            for k in range(MT - 2, KT):
                ld(hs, MT + k, hT)
                ld(hs, k, hT)
            # z0/z1 weights and fp16 x stream behind the r-phase needs
            ws_z0 = [None] * 6
            ws_z1 = [None] * 6
            for base in (0, 3):
                wload(ws_z0, 0, 0, base + 1)
                wload(ws_z0, 0, 0, base)
            for k in range(4):
                ld(xs, MT + k, xT)
                ld(xs, k, xT)
            for base in (0, 3):
                wload(ws_z1, 0, 1, base + 1)
                wload(ws_z1, 0, 1, base)
            for k in range(4, KT):
                ld(xs, MT + k, xT)
                ld(xs, k, xT)

            class T:
                def __init__(self, kind, m, cols, ws=None, tail=False):
                    self.kind, self.m, self.cols, self.tail = kind, m, cols, tail
                    # alloc order (A, B, K1) matches in-half issue order so
                    # bank recycling waits line up with the combine stts
                    self.A = psp.tile([P, N], F32, tag="ps")
                    self.B = psp.tile([P, N], F32, tag="ps")
                    self.K1 = psp.tile([P, N], F32, tag="ps")
                    if ws is not None:
                        self.ws = ws
                        if kind != "r":
                            for base in (0, 3):
                                wb = wts.tile([P, MCOLS], F16, tag="w")
                                nc.vector.scalar_tensor_tensor(
                                    wb[:], ws[base][:], -2.0,
                                    ws[base + 1][:],
                                    op0=OP.mult, op1=OP.subtract)
                                ws[base + 2] = wb
                        return
                    if kind == "r":
                        self.ws = [None] * 6
                        for j in (1, 2, 0, 4, 3, 5):
                            w8load(self.ws, m, j)
                    else:
                        # stream R/(I-R) per half, derive -(I+R) = -2R-(I-R)
                        # on the DVE (saves a third of the weight DMA)
                        gi = 0 if kind == "z" else 1
                        self.ws = [None] * 6
                        for base in (0, 3):
                            wload(self.ws, gi, m, base + 1)
                            wload(self.ws, gi, m, base)
                            wb = wts.tile([P, MCOLS], F16, tag="w")
                            nc.vector.scalar_tensor_tensor(
                                wb[:], self.ws[base][:], -2.0,
                                self.ws[base + 1][:],
                                op0=OP.mult, op1=OP.subtract)
                            self.ws[base + 2] = wb

            def dr_chain(t, ps, wj, buf, off, start):
                c = t.cols
                for j in range(KT // 2):
                    nc.tensor.matmul(
                        ps[:, c], t.ws[wj][:, 2 * j:2 * j + 2, :],
                        buf[:, off + 2 * j:off + 2 * j + 2, c],
                        start=(start and j == 0),
                        stop=(not start and j == KT // 2 - 1),
                        perf_mode=DR)

            def f16_chain(t, ps, wj, buf, off, start):
                c = t.cols
                for k in range(KT):
                    nc.tensor.matmul(
                        ps[:, c], t.ws[wj][:, k * P:(k + 1) * P],
                        buf[:, off + k, c],
                        start=(start and k == 0),
                        stop=(not start and k == KT - 1))

            def in_half(t, kmajor=False):
                c = t.cols
                if t.kind == "r":
                    if kmajor:
                        # consume (xi8[2j..], xr8[2j..]) at DMA arrival rate
                        for j in range(KT // 2):
                            nc.tensor.matmul(
                                t.A[:, c], t.ws[1][:, 2 * j:2 * j + 2, :],
                                x8[:, MT + 2 * j:MT + 2 * j + 2, c],
                                start=(j == 0), stop=False, perf_mode=DR)
                            nc.tensor.matmul(
                                t.B[:, c], t.ws[2][:, 2 * j:2 * j + 2, :],
                                x8[:, 2 * j:2 * j + 2, c],
                                start=(j == 0), stop=False, perf_mode=DR)
                        dr_chain(t, t.K1, 0, xsum8, 0, True)
                    else:
                        dr_chain(t, t.A, 1, x8, MT, True)
                        dr_chain(t, t.B, 2, x8, 0, True)
                        dr_chain(t, t.K1, 0, xsum8, 0, True)
                else:
                    f16_chain(t, t.A, 1, xs, MT, True)
                    f16_chain(t, t.B, 2, xs, 0, True)
                    f16_chain(t, t.K1, 0, xsum, 0, True)

            def rec_and_combine(t):
                c, m = t.cols, t.m
                if t.kind == "r":
                    dr_chain(t, t.A, 4, h8, MT, False)
                    dr_chain(t, t.K1, 3, hsum8, 0, False)
                else:
                    mv, sm = (rh, rhsum) if t.kind == "h" else (hs, hsum)
                    f16_chain(t, t.A, 4, mv, MT, False)
                    f16_chain(t, t.K1, 3, sm, 0, False)
                # ALU ops may read only one PSUM operand: stage K1 in SBUF
                # (also releases its bank early)
                k1sb = act.tile([P, N], F32, tag="k1")
                nc.scalar.copy(k1sb[:, c], t.K1[:, c])
                pre_r = act.tile([P, N], F16, tag="pre")
                nc.vector.scalar_tensor_tensor(
                    pre_r[:, c], k1sb[:, c], 1.0, t.A[:, c],
                    op0=OP.mult, op1=OP.add)
                if t.kind == "r":
                    dr_chain(t, t.B, 5, h8, 0, False)
                else:
                    f16_chain(t, t.B, 5, mv, 0, False)
                pre_i = act.tile([P, N], F16, tag="pre")
                for ch in ((slice(0, N // 2), slice(N // 2, N))
                           if t.tail else (c,)):
                    nc.vector.scalar_tensor_tensor(
                        pre_i[:, ch], k1sb[:, ch], 1.0, t.B[:, ch],
                        op0=OP.mult, op1=OP.add)

                if t.kind == "z":
                    for col, pre in ((m, pre_r), (MT + m, pre_i)):
                        nc.scalar.activation(zs[:, col, c], pre[:, c], AF.Relu,
                                             bias=bz_sb[:, col:col + 1],
                                             scale=0.2)
                elif t.kind == "r":
                    for col, pre in ((m, pre_r), (MT + m, pre_i)):
                        rr = act.tile([P, N], F16, tag="rr")
                        nc.scalar.activation(rr[:, c], pre[:, c], AF.Relu,
                                             bias=br_sb[:, col:col + 1],
                                             scale=0.2 / S8)
                        nc.vector.scalar_tensor_tensor(
                            rh[:, col, c], rr[:, c], 1.0, hs[:, col, c],
                            op0=OP.min, op1=OP.mult)
                    nc.vector.tensor_tensor(rhsum[:, m, c], rh[:, m, c],
                                            rh[:, MT + m, c], OP.add)
                elif t.tail:
                    # short tail: o = u + w*t with u = min(z,1)*h and
                    # w = relu(1-z) precomputed off the critical path
                    for part, (col, pre) in enumerate(((m, pre_r),
                                                       (MT + m, pre_i))):
                        chunks = ((slice(0, N // 2), slice(N // 2, N))
                                  if part else (c,))
                        for ch in chunks:
                            t_ = act.tile([P, N], F16, tag="t")
                            nc.scalar.activation(t_[:, ch], pre[:, ch],
                                                 AF.Tanh,
                                                 bias=bh_sb[:, col:col + 1])
                            v = act.tile([P, N], F16, tag="d")
                            nc.vector.tensor_tensor(
                                v[:, ch], wpre[part][:, ch], t_[:, ch],
                                OP.mult)
                            o = act.tile([P, N], F16, tag="o")
                            nc.vector.tensor_tensor(o[:, ch], v[:, ch],
                                                    upre[part][:, ch], OP.add)
                            nc.sync.dma_start(oT[col * P:(col + 1) * P, ch],
                                              o[:, ch])
                else:
                    for col, pre in ((m, pre_r), (MT + m, pre_i)):
                        t_ = act.tile([P, N], F16, tag="t")
                        nc.scalar.activation(t_[:, c], pre[:, c], AF.Tanh,
                                             bias=bh_sb[:, col:col + 1])
                        d = act.tile([P, N], F16, tag="d")
                        nc.vector.scalar_tensor_tensor(
                            d[:, c], t_[:, c], -1.0, hs[:, col, c],
                            op0=OP.mult, op1=OP.add)
                        e = act.tile([P, N], F16, tag="e")
                        nc.vector.scalar_tensor_tensor(
                            e[:, c], zs[:, col, c], 1.0, d[:, c],
                            op0=OP.min, op1=OP.mult)
                        o = act.tile([P, N], F16, tag="o")
                        nc.vector.tensor_tensor(o[:, c], e[:, c], t_[:, c],
                                                OP.add)
                        nc.sync.dma_start(oT[col * P:(col + 1) * P, c],
                                          o[:, c])

            upre = [res.tile([P, N], F16, tag="u0", name="u0"),
                    res.tile([P, N], F16, tag="u1", name="u1")]
            wpre = [res.tile([P, N], F16, tag="w0", name="w0"),
                    res.tile([P, N], F16, tag="w1", name="w1")]

            full = slice(0, N)
            plan = ([("r", m) for m in range(MT)]
                    + [("z", m) for m in range(MT)]
                    + [("h", m) for m in range(MT)])

            prev = None
            for kind, m in plan:
                if (kind, m) == ("z", 0):
                    # sums for the fp16 k1 chains; emitted here so the DVE
                    # doesn't block the r combines while x/h stream in
                    for k in range(KT):
                        nc.vector.tensor_tensor(xsum[:, k, :], xs[:, k, :],
                                                xs[:, MT + k, :], OP.add)
                    for k in range(KT):
                        nc.vector.tensor_tensor(hsum[:, k, :], hs[:, k, :],
                                                hs[:, MT + k, :], OP.add)
                tail = kind == "h" and m == MT - 1
                if tail:
                    for part, col in enumerate((MT - 1, 2 * MT - 1)):
                        nc.vector.scalar_tensor_tensor(
                            upre[part][:], zs[:, col, :], 1.0, hs[:, col, :],
                            op0=OP.min, op1=OP.mult)
                        nc.scalar.activation(wpre[part][:], zs[:, col, :],
                                             AF.Relu, bias=1.0, scale=-1.0)
                pre_ws = (ws_r[m] if kind == "r" else
                          {("z", 0): ws_z0, ("z", 1): ws_z1}.get((kind, m)))
                t = T(kind, m, full, ws=pre_ws, tail=tail)
                in_half(t, kmajor=(prev is None))
                if prev is not None:
                    rec_and_combine(prev)
                prev = t
            rec_and_combine(prev)

    nc.compile()
    return nc


def _tiles(mat):
    # (1024, 1024) -> [m, k, 128, 128] tile array
    return mat.reshape(KT, P, MT, P).transpose(2, 0, 1, 3)


def _gate_mats(real_kernel, imaginary_kernel, real_recurrent_kernel,
               imaginary_recurrent_kernel, g):
    def gate(Wmat):
        return np.asarray(Wmat[:, g * U:(g + 1) * U], dtype=np.float32)

    R, I = gate(real_kernel), gate(imaginary_kernel)
    RR, IR = gate(real_recurrent_kernel), gate(imaginary_recurrent_kernel)
    return [R, I - R, -(I + R), RR, IR - RR, -(IR + RR)]


def _blob16(mats):
    arr = np.stack([_tiles(m) for m in mats])   # [6, m, k, p, c]
    arr = arr.transpose(1, 0, 3, 2, 4)          # [m, mat, p, k, c]
    return arr.reshape(MT, 6, P, MCOLS).astype(np.float16)


def _blob8(mats):
    arr = np.stack([_tiles(m) for m in mats])
    arr = arr.transpose(1, 0, 3, 2, 4) * S8     # [m, mat, p, k, c]
    return np.ascontiguousarray(arr).astype(ml_dtypes.float8_e4m3)


def prepare_in_maps(inputs, h_tm1, real_kernel, imaginary_kernel,
                    real_recurrent_kernel, imaginary_recurrent_kernel,
                    real_bias, imaginary_bias):
    inputs = np.asarray(inputs, dtype=np.float32)
    h_tm1 = np.asarray(h_tm1, dtype=np.float32)

    def mats(g):
        return _gate_mats(real_kernel, imaginary_kernel,
                          real_recurrent_kernel, imaginary_recurrent_kernel, g)

    w16_np = np.ascontiguousarray(np.stack([_blob16(mats(0)),
                                            _blob16(mats(2))]))
    w8_np = _blob8(mats(1))

    def cat_bias(g):
        return np.concatenate([
            np.asarray(real_bias[g * U:(g + 1) * U], dtype=np.float32),
            np.asarray(imaginary_bias[g * U:(g + 1) * U], dtype=np.float32),
        ])

    def pcols(v):  # [2U] -> [P, 2MT]
        return np.ascontiguousarray(v.reshape(2 * MT, P).T)

    bz_np = pcols(0.2 * cat_bias(0) + 0.5)
    br_np = pcols(0.2 * cat_bias(1) + 0.5)
    bh_np = pcols(cat_bias(2))
    wz_np = np.zeros((P, P + N), dtype=np.float16)

    in_maps = []
    for c in range(NCORES):
        sl = slice(c * N, (c + 1) * N)
        xT = inputs[sl].T.astype(np.float16)
        hT = h_tm1[sl].T.astype(np.float16)
        xsd = (inputs[sl, :D].T + inputs[sl, D:].T).astype(np.float16)
        hsd = (h_tm1[sl, :D].T + h_tm1[sl, D:].T).astype(np.float16)
        in_maps.append({
            "xT": xT, "hT": hT,
            "x8d": xT.astype(ml_dtypes.float8_e4m3),
            "h8d": hT.astype(ml_dtypes.float8_e4m3),
            "xs8d": xsd.astype(ml_dtypes.float8_e4m3),
            "hs8d": hsd.astype(ml_dtypes.float8_e4m3),
            "w16": w16_np, "w8": w8_np,
            "bz": bz_np, "br": br_np, "bh": bh_np, "wz": wz_np,
        })
    return in_maps


def get_nc():
    if "nc" not in _CACHE:
        _CACHE["nc"] = _build()
    return _CACHE["nc"]


def gather(results):
    out = np.empty((B, F), dtype=np.float32)
    for c in range(NCORES):
        out[c * N:(c + 1) * N] = results[c]["oT"].T.astype(np.float32)
    return out


def kernel(**inputs):
    nc = get_nc()
    in_maps = prepare_in_maps(**inputs)
    res = run_bass_kernel_spmd(nc, in_maps, list(range(NCORES)))
    return gather(res.results)


# revision 35
# speedup vs baseline: 1.2568x; 1.0560x over previous
"""CGRU cell on 8 Trainium2 NeuronCores.

Strategy: data-parallel over the batch dim (4096 -> 8 x 512), zero
cross-core communication, weights replicated.  On-core compute runs in
transposed space ([feature, batch]): W tiles are the stationary matmul
operand, x^T/h^T tiles [128, 512] the moving operand.

The complex "cat kernel" [[R, -I], [I, R]] is evaluated with Gauss's
3-multiplication trick instead of 4 real matmul chains:
    k1   = (xr + xi) @ R        (shared by both outputs)
    real = k1 + xi @ (I - R)
    imag = k1 + xr @ (-I - R)
which cuts PE work by 25%.  The combine (k1 + A) is a DVE
scalar_tensor_tensor (k1 staged to SBUF first - ALU ops may read only
one PSUM operand), fused with the per-feature bias; hard-sigmoid
scale/clip ride the scalar-engine activation.

The r gate (least error-sensitive: its error passes through another
matmul and a tanh) runs in fp8-e4m3 with DoubleRow perf mode: each
matmul contracts TWO k-tiles in the same 216 ns a fp16 matmul spends on
one (157 TF/s).  Its weights are host-scaled by S8=64 to stay out of
the fp8 subnormal range; the activation's scale folds 0.2/S8 back out.
z and hh stay fp16: fp8 there breaks the 2e-2 error budget (measured).

Phase order r -> z -> hh: the r phase's inputs (host-quantized fp8 x/h
and their sums) total ~3 MB, so the PE starts almost immediately while
the 20+ MB of fp16 activations/weights stream in its shadow - the
startup is otherwise HBM-bound.  fp16 triples stream R/(I-R) per half
and derive -(I+R) = -2R - (I-R) on the DVE, saving a third of the
weight DMA.  Everything accumulates in fp32 PSUM; outputs store as
fp16 (upcast on host).
"""

import numpy as np
import ml_dtypes

import concourse.bass as bass
import concourse.mybir as mybir
import concourse.tile as tile
from concourse import bacc
from concourse.bass_utils import run_bass_kernel_spmd

B, D, U = 4096, 1024, 1024
NCORES = 8
N = B // NCORES          # batch rows per core (moving free dim)
P = 128                  # partition size
KT = D // P              # 8 k-tiles per complex half
MT = U // P              # 8 m-tiles per complex half
F = 2 * U                # 2048 features
MCOLS = KT * P           # 1024 cols per per-matrix weight tile
S8 = 64.0                # fp8 weight pre-scale for the r gate
WARM = 14

F16 = mybir.dt.float16
F32 = mybir.dt.float32
F8 = mybir.dt.float8e4
AF = mybir.ActivationFunctionType
OP = mybir.AluOpType
DR = mybir.MatmulPerfMode.DoubleRow

_CACHE = {}


def _build():
    nc = bacc.Bacc("TRN2", target_bir_lowering=False, debug=False)

    xT = nc.dram_tensor("xT", [F, N], F16, kind="ExternalInput")
    hT = nc.dram_tensor("hT", [F, N], F16, kind="ExternalInput")
    x8d = nc.dram_tensor("x8d", [F, N], F8, kind="ExternalInput")
    h8d = nc.dram_tensor("h8d", [F, N], F8, kind="ExternalInput")
    xs8d = nc.dram_tensor("xs8d", [D, N], F8, kind="ExternalInput")
    hs8d = nc.dram_tensor("hs8d", [D, N], F8, kind="ExternalInput")
    # z/hh weights: [gate, m, mat, part, k*128] fp16,
    # mats: R, I-R, -(I+R), RR, IR-RR, -(IR+RR)
    w16 = nc.dram_tensor("w16", [2, MT, 6, P, MCOLS], F16, kind="ExternalInput")
    # r weights: same six mats, * S8, [m, mat, part, k, 128] fp8 (DoubleRow)
    w8 = nc.dram_tensor("w8", [MT, 6, P, KT, P], F8, kind="ExternalInput")
    bz = nc.dram_tensor("bz", [P, 2 * MT], F32, kind="ExternalInput")
    br = nc.dram_tensor("br", [P, 2 * MT], F32, kind="ExternalInput")
    bh = nc.dram_tensor("bh", [P, 2 * MT], F32, kind="ExternalInput")
    oT = nc.dram_tensor("oT", [F, N], F16, kind="ExternalOutput")

    with tile.TileContext(nc) as tc:
        with (
            tc.tile_pool(name="res", bufs=1) as res,
            tc.tile_pool(name="wts", bufs=18) as wts,
            tc.tile_pool(name="w8p", bufs=18) as w8p,
            tc.tile_pool(name="act", bufs=4) as act,
            tc.tile_pool(name="ps", bufs=7, space="PSUM") as psp,
            tc.tile_pool(name="wm", bufs=1, space="PSUM") as wmp,
        ):
            # PE warmup: keeps the HAM activity window busy while the
            # first DMAs land.
            wsrc = res.tile([P, P], F16, tag="wsrc")
            dmov = res.tile([P, N], F16, tag="dmov")
            nc.vector.memset(wsrc[:], 0.0)
            nc.vector.memset(dmov[:], 0.0)
            wps = wmp.tile([P, N], F32, tag="warm")
            for _ in range(WARM):
                nc.tensor.matmul(wps[:], wsrc[:], dmov[:], start=True, stop=True)

            xs = res.tile([P, 2 * MT, N], F16, tag="xs")      # xr 0..7, xi 8..15
            hs = res.tile([P, 2 * MT, N], F16, tag="hs")
            xsum = res.tile([P, KT, N], F16, tag="xsum")      # xr + xi
            hsum = res.tile([P, KT, N], F16, tag="hsum")
            x8 = res.tile([P, 2 * MT, N], F8, tag="x8")
            h8 = res.tile([P, 2 * MT, N], F8, tag="h8")
            xsum8 = res.tile([P, KT, N], F8, tag="xsum8")
            hsum8 = res.tile([P, KT, N], F8, tag="hsum8")
            rh = res.tile([P, 2 * MT, N], F16, tag="rh")      # min(r,1)*h
            rhsum = res.tile([P, KT, N], F16, tag="rhsum")
            zs = res.tile([P, 2 * MT, N], F16, tag="zs")
            bz_sb = res.tile([P, 2 * MT], F32, tag="bz")
            br_sb = res.tile([P, 2 * MT], F32, tag="br")
            bh_sb = res.tile([P, 2 * MT], F32, tag="bh")

            nc.scalar.dma_start(br_sb[:], br[:])
            nc.scalar.dma_start(bz_sb[:], bz[:])
            nc.scalar.dma_start(bh_sb[:], bh[:])

            def wload(ws, gi, m, j):
                wt = wts.tile([P, MCOLS], F16, tag="w")
                nc.sync.dma_start(wt[:], w16[gi, m, j])
                ws[j] = wt

            def w8load(ws, m, j):
                wt = w8p.tile([P, KT, P], F8, tag="w8")
                nc.sync.dma_start(wt[:], w8[m, j])
                ws[j] = wt

            def ld(dst, j, src):
                nc.sync.dma_start(dst[:, j, :], src[j * P:(j + 1) * P, :])

            # --- startup: one JIT-ordered DMA stream on the sync queue.
            # The fp8 r-phase operands and r0/r1 weights go first (the PE
            # starts on them ~7us in); everything fp16 streams behind. ---
            ws_r = [[None] * 6 for _ in range(MT)]
            ws_z0 = [None] * 6
            ws_z1 = [None] * 6

            def xpair(k):
                ld(xs, MT + k, xT)
                ld(xs, k, xT)

            def hpair(k):
                ld(hs, MT + k, hT)
                ld(hs, k, hT)

            xpair(0)
            xpair(1)
            wload(ws_z0, 0, 0, 1)
            xpair(2)
            wload(ws_z0, 0, 0, 0)
            xpair(3)
            xpair(4)
            wload(ws_z1, 0, 1, 1)
            xpair(5)
            wload(ws_z1, 0, 1, 0)
            xpair(6)
            xpair(7)
            hpair(0)
            hpair(1)
            wload(ws_z0, 0, 0, 4)
            hpair(2)
            wload(ws_z0, 0, 0, 3)
            hpair(3)
            hpair(4)
            wload(ws_z1, 0, 1, 4)
            hpair(5)
            wload(ws_z1, 0, 1, 3)
            hpair(6)
            hpair(7)

            for wsx in (ws_z0, ws_z1):
                for base in (0, 3):
                    wbp = wts.tile([P, MCOLS], F16, tag="w", name="wbp")
                    nc.vector.scalar_tensor_tensor(
                        wbp[:], wsx[base][:], -2.0, wsx[base + 1][:],
                        op0=OP.mult, op1=OP.subtract)
                    wsx[base + 2] = wbp

            def emit_fp8_stream():
                # r-phase operands + all its weights, streamed in the
                # z-phase's shadow
                for k in range(KT):
                    ld(x8, MT + k, x8d)
                    ld(x8, k, x8d)
                for k in range(KT):
                    ld(xsum8, k, xs8d)
                for k in range(KT):
                    ld(h8, MT + k, h8d)
                    ld(h8, k, h8d)
                for k in range(KT):
                    ld(hsum8, k, hs8d)
                for m in range(MT):
                    for j in (1, 2, 0, 4, 3, 5):
                        w8load(ws_r[m], m, j)

            class T:
                def __init__(self, kind, m, cols, ws=None, tail=False):
                    self.kind, self.m, self.cols, self.tail = kind, m, cols, tail
                    # alloc order (A, B, K1) matches in-half issue order so
                    # bank recycling waits line up with the combine stts
                    self.A = psp.tile([P, N], F32, tag="ps")
                    self.B = psp.tile([P, N], F32, tag="ps")
                    self.K1 = psp.tile([P, N], F32, tag="ps")
                    if ws is not None:
                        self.ws = ws
                        return
                    if kind == "r":
                        self.ws = [None] * 6
                        for j in (1, 2, 0, 4, 3, 5):
                            w8load(self.ws, m, j)
                    else:
                        # stream R/(I-R) per half, derive -(I+R) = -2R-(I-R)
                        # on the DVE (saves a third of the weight DMA)
                        gi = 0 if kind == "z" else 1
                        self.ws = [None] * 6
                        for base in (0, 3):
                            wload(self.ws, gi, m, base + 1)
                            wload(self.ws, gi, m, base)
                            wb = wts.tile([P, MCOLS], F16, tag="w")
                            nc.vector.scalar_tensor_tensor(
                                wb[:], self.ws[base][:], -2.0,
                                self.ws[base + 1][:],
                                op0=OP.mult, op1=OP.subtract)
                            self.ws[base + 2] = wb

            def dr_chain(t, ps, wj, buf, off, start):
                c = t.cols
                for j in range(KT // 2):
                    nc.tensor.matmul(
                        ps[:, c], t.ws[wj][:, 2 * j:2 * j + 2, :],
                        buf[:, off + 2 * j:off + 2 * j + 2, c],
                        start=(start and j == 0),
                        stop=(not start and j == KT // 2 - 1),
                        perf_mode=DR)

            def f16_chain(t, ps, wj, buf, off, start):
                c = t.cols
                for k in range(KT):
                    nc.tensor.matmul(
                        ps[:, c], t.ws[wj][:, k * P:(k + 1) * P],
                        buf[:, off + k, c],
                        start=(start and k == 0),
                        stop=(not start and k == KT - 1))

            def in_half(t, kmajor=False):
                c = t.cols
                if t.kind == "r":
                    if kmajor:
                        # consume (xi8[2j..], xr8[2j..]) at DMA arrival rate
                        for j in range(KT // 2):
                            nc.tensor.matmul(
                                t.A[:, c], t.ws[1][:, 2 * j:2 * j + 2, :],
                                x8[:, MT + 2 * j:MT + 2 * j + 2, c],
                                start=(j == 0), stop=False, perf_mode=DR)
                            nc.tensor.matmul(
                                t.B[:, c], t.ws[2][:, 2 * j:2 * j + 2, :],
                                x8[:, 2 * j:2 * j + 2, c],
                                start=(j == 0), stop=False, perf_mode=DR)
                        dr_chain(t, t.K1, 0, xsum8, 0, True)
                    else:
                        dr_chain(t, t.A, 1, x8, MT, True)
                        dr_chain(t, t.B, 2, x8, 0, True)
                        dr_chain(t, t.K1, 0, xsum8, 0, True)
                else:
                    f16_chain(t, t.A, 1, xs, MT, True)
                    f16_chain(t, t.B, 2, xs, 0, True)
                    f16_chain(t, t.K1, 0, xsum, 0, True)

            def rec_and_combine(t):
                c, m = t.cols, t.m
                if t.kind == "r":
                    dr_chain(t, t.A, 4, h8, MT, False)
                    dr_chain(t, t.K1, 3, hsum8, 0, False)
                else:
                    mv, sm = (rh, rhsum) if t.kind == "h" else (hs, hsum)
                    f16_chain(t, t.A, 4, mv, MT, False)
                    f16_chain(t, t.K1, 3, sm, 0, False)
                # ALU ops may read only one PSUM operand: stage K1 in SBUF
                # (also releases its bank early)
                k1sb = act.tile([P, N], F32, tag="k1")
                nc.scalar.copy(k1sb[:, c], t.K1[:, c])
                pre_r = act.tile([P, N], F16, tag="pre")
                nc.vector.scalar_tensor_tensor(
                    pre_r[:, c], k1sb[:, c], 1.0, t.A[:, c],
                    op0=OP.mult, op1=OP.add)
                if t.kind == "r":
                    dr_chain(t, t.B, 5, h8, 0, False)
                else:
                    f16_chain(t, t.B, 5, mv, 0, False)
                pre_i = act.tile([P, N], F16, tag="pre")
                for ch in ((slice(0, N // 2), slice(N // 2, N))
                           if t.tail else (c,)):
                    nc.vector.scalar_tensor_tensor(
                        pre_i[:, ch], k1sb[:, ch], 1.0, t.B[:, ch],
                        op0=OP.mult, op1=OP.add)

                if t.kind == "z":
                    for col, pre in ((m, pre_r), (MT + m, pre_i)):
                        nc.scalar.activation(zs[:, col, c], pre[:, c], AF.Relu,
                                             bias=bz_sb[:, col:col + 1],
                                             scale=0.2)
                elif t.kind == "r":
                    for col, pre in ((m, pre_r), (MT + m, pre_i)):
                        rr = act.tile([P, N], F16, tag="rr")
                        nc.scalar.activation(rr[:, c], pre[:, c], AF.Relu,
                                             bias=br_sb[:, col:col + 1],
                                             scale=0.2 / S8)
                        nc.vector.scalar_tensor_tensor(
                            rh[:, col, c], rr[:, c], 1.0, hs[:, col, c],
                            op0=OP.min, op1=OP.mult)
                    nc.vector.tensor_tensor(rhsum[:, m, c], rh[:, m, c],
                                            rh[:, MT + m, c], OP.add)
                elif t.tail:
                    # short tail: o = u + w*t with u = min(z,1)*h and
                    # w = relu(1-z) precomputed off the critical path
                    for part, (col, pre) in enumerate(((m, pre_r),
                                                       (MT + m, pre_i))):
                        chunks = ((slice(0, N // 2), slice(N // 2, N))
                                  if part else (c,))
                        for ch in chunks:
                            t_ = act.tile([P, N], F16, tag="t")
                            nc.scalar.activation(t_[:, ch], pre[:, ch],
                                                 AF.Tanh,
                                                 bias=bh_sb[:, col:col + 1])
                            v = act.tile([P, N], F16, tag="d")
                            nc.vector.tensor_tensor(
                                v[:, ch], wpre[part][:, ch], t_[:, ch],
                                OP.mult)
                            o = act.tile([P, N], F16, tag="o")
                            nc.vector.tensor_tensor(o[:, ch], v[:, ch],
                                                    upre[part][:, ch], OP.add)
                            nc.sync.dma_start(oT[col * P:(col + 1) * P, ch],
                                              o[:, ch])
                else:
                    for col, pre in ((m, pre_r), (MT + m, pre_i)):
                        t_ = act.tile([P, N], F16, tag="t")
                        nc.scalar.activation(t_[:, c], pre[:, c], AF.Tanh,
                                             bias=bh_sb[:, col:col + 1])
                        d = act.tile([P, N], F16, tag="d")
                        nc.vector.scalar_tensor_tensor(
                            d[:, c], t_[:, c], -1.0, hs[:, col, c],
                            op0=OP.mult, op1=OP.add)
                        e = act.tile([P, N], F16, tag="e")
                        nc.vector.scalar_tensor_tensor(
                            e[:, c], zs[:, col, c], 1.0, d[:, c],
                            op0=OP.min, op1=OP.mult)
                        o = act.tile([P, N], F16, tag="o")
                        nc.vector.tensor_tensor(o[:, c], e[:, c], t_[:, c],
                                                OP.add)
                        nc.sync.dma_start(oT[col * P:(col + 1) * P, c],
                                          o[:, c])

            upre = [res.tile([P, N], F16, tag="u0", name="u0"),
                    res.tile([P, N], F16, tag="u1", name="u1")]
            wpre = [res.tile([P, N], F16, tag="w0", name="w0"),
                    res.tile([P, N], F16, tag="w1", name="w1")]

            full = slice(0, N)
            plan = ([("z", m) for m in range(MT)]
                    + [("r", m) for m in range(MT)]
                    + [("h", m) for m in range(MT)])

            for k in range(KT):
                nc.vector.tensor_tensor(xsum[:, k, :], xs[:, k, :],
                                        xs[:, MT + k, :], OP.add)
            for k in range(KT):
                nc.vector.tensor_tensor(hsum[:, k, :], hs[:, k, :],
                                        hs[:, MT + k, :], OP.add)

            prev = None
            for kind, m in plan:
                if (kind, m) == ("z", 5):
                    emit_fp8_stream()
                tail = kind == "h" and m == MT - 1
                if tail:
                    for part, col in enumerate((MT - 1, 2 * MT - 1)):
                        nc.vector.scalar_tensor_tensor(
                            upre[part][:], zs[:, col, :], 1.0, hs[:, col, :],
                            op0=OP.min, op1=OP.mult)
                        nc.scalar.activation(wpre[part][:], zs[:, col, :],
                                             AF.Relu, bias=1.0, scale=-1.0)
                pre_ws = (ws_r[m] if kind == "r" else
                          {("z", 0): ws_z0, ("z", 1): ws_z1}.get((kind, m)))
                if kind == "r" and any(w is None for w in pre_ws):
                    raise AssertionError("r weights not preloaded")
                t = T(kind, m, full, ws=pre_ws, tail=tail)
                in_half(t, kmajor=(prev is None))
                if prev is not None:
                    rec_and_combine(prev)
                prev = t
            rec_and_combine(prev)

    nc.compile()
    return nc


def _tiles(mat):
    # (1024, 1024) -> [m, k, 128, 128] tile array
    return mat.reshape(KT, P, MT, P).transpose(2, 0, 1, 3)


def _gate_mats(real_kernel, imaginary_kernel, real_recurrent_kernel,
               imaginary_recurrent_kernel, g):
    def gate(Wmat):
        return np.asarray(Wmat[:, g * U:(g + 1) * U], dtype=np.float32)

    R, I = gate(real_kernel), gate(imaginary_kernel)
    RR, IR = gate(real_recurrent_kernel), gate(imaginary_recurrent_kernel)
    return [R, I - R, -(I + R), RR, IR - RR, -(IR + RR)]


def _blob16(mats):
    arr = np.stack([_tiles(m) for m in mats])   # [6, m, k, p, c]
    arr = arr.transpose(1, 0, 3, 2, 4)          # [m, mat, p, k, c]
    return arr.reshape(MT, 6, P, MCOLS).astype(np.float16)


def _blob8(mats):
    arr = np.stack([_tiles(m) for m in mats])
    arr = arr.transpose(1, 0, 3, 2, 4) * S8     # [m, mat, p, k, c]
    return np.ascontiguousarray(arr).astype(ml_dtypes.float8_e4m3)


def prepare_in_maps(inputs, h_tm1, real_kernel, imaginary_kernel,
                    real_recurrent_kernel, imaginary_recurrent_kernel,
                    real_bias, imaginary_bias):
    inputs = np.asarray(inputs, dtype=np.float32)
    h_tm1 = np.asarray(h_tm1, dtype=np.float32)

    def mats(g):
        return _gate_mats(real_kernel, imaginary_kernel,
                          real_recurrent_kernel, imaginary_recurrent_kernel, g)

    w16_np = np.ascontiguousarray(np.stack([_blob16(mats(0)),
                                            _blob16(mats(2))]))
    w8_np = _blob8(mats(1))

    def cat_bias(g):
        return np.concatenate([
            np.asarray(real_bias[g * U:(g + 1) * U], dtype=np.float32),
            np.asarray(imaginary_bias[g * U:(g + 1) * U], dtype=np.float32),
        ])

    def pcols(v):  # [2U] -> [P, 2MT]
        return np.ascontiguousarray(v.reshape(2 * MT, P).T)

    bz_np = pcols(0.2 * cat_bias(0) + 0.5)
    br_np = pcols(0.2 * cat_bias(1) + 0.5)
    bh_np = pcols(cat_bias(2))

    in_maps = []
    for c in range(NCORES):
        sl = slice(c * N, (c + 1) * N)
        xT = inputs[sl].T.astype(np.float16)
        hT = h_tm1[sl].T.astype(np.float16)
        xsd = (inputs[sl, :D].T + inputs[sl, D:].T).astype(np.float16)
        hsd = (h_tm1[sl, :D].T + h_tm1[sl, D:].T).astype(np.float16)
        in_maps.append({
            "xT": xT, "hT": hT,
            "x8d": xT.astype(ml_dtypes.float8_e4m3),
            "h8d": hT.astype(ml_dtypes.float8_e4m3),
            "xs8d": xsd.astype(ml_dtypes.float8_e4m3),
            "hs8d": hsd.astype(ml_dtypes.float8_e4m3),
            "w16": w16_np, "w8": w8_np,
            "bz": bz_np, "br": br_np, "bh": bh_np,
        })
    return in_maps


def get_nc():
    if "nc" not in _CACHE:
        _CACHE["nc"] = _build()
    return _CACHE["nc"]


def gather(results):
    out = np.empty((B, F), dtype=np.float32)
    for c in range(NCORES):
        out[c * N:(c + 1) * N] = results[c]["oT"].T.astype(np.float32)
    return out


def kernel(**inputs):
    nc = get_nc()
    in_maps = prepare_in_maps(**inputs)
    res = run_bass_kernel_spmd(nc, in_maps, list(range(NCORES)))
    return gather(res.results)
